# revision 3
# baseline (speedup 1.0000x reference)
"""Trainium2 Bass kernel for nn_DynamicFc (per-sample dynamic MLP).

Data-parallel over 8 cores (batch 8192 -> 8 x 1024), 8 tiles of 128/core.

v2: host-side transposed input layouts (plain dense DMA loads, no xbar
transpose), CCE-accumulate DMA absorbs the widest tree-reduce levels,
chunk routing across DVE/Scalar/GPSIMD retuned.

Per 128-sample tile (natural [batch, feat] layout):
  psum_fl = f @ [Wf.T | Wf.T@B1] + [bf | bf@B1]        (PE, 160 cols)
  pf_lowT = (Wpf.T).T-chunks @ pfT-chunks + bpf        (PE, transposed [j,b])
  p1 = pf_low @ W1 (cols (l,m)), p2 = pf_low @ W2 (cols (m,l))   (PE)
  prod1 = p1 * f_low  (pair-duplicated broadcast; per-chunk routes)
  h = relu(tree_reduce_l(prod1) + h_bias)              (DMA/DVE/GPS tree)
  prod2 = p2 * h; g = tree_reduce_m(prod2)             (same scheme)
  out = gT.T @ fin1 + [hT].T @ B2 + r16                (PE + DVE adds)
Residual r16 = (f+pf+b2) in bf16 from host; output stored bf16.
"""

import os
import sys

import numpy as np

for _p in ("/opt/trn_rl_repo", "/root/.axon_site/_ro/trn_rl_repo"):
    if os.path.isdir(_p) and _p not in sys.path:
        sys.path.insert(0, _p)

import bass_rust
import concourse.bass as bass
import concourse.mybir as mybir
import concourse.tile as tile
from concourse.bass_utils import run_bass_kernel_spmd
from concourse.masks import make_identity

import ml_dtypes

N_CORES = 8
B, D, LOW, MID = 8192, 1024, 128, 32
DIM = LOW * MID  # 4096
SHARD = B // N_CORES  # 1024
TILE_B = 128
NT = SHARD // TILE_B  # 8
NCHUNK = D // 128  # 8

F32 = mybir.dt.float32
CDT = mybir.dt.bfloat16
NP_CDT = ml_dtypes.bfloat16

_CACHED = {}

# per-chunk routing for the params first-touch multiply (4 chunks of 1024
# per half): "V" = DVE direct from psum, "SV" = scalar copy psum->sbuf bf16
# then DVE multiply, "SG" = scalar copy then gpsimd multiply.
ROUTE1 = ("V", "V", "SV", "SG")
ROUTE2 = ("V", "V", "SV", "SG")
# tree level engine by fold half-width w (contiguous halving folds):
# tree1 w: 2048,1024,512,256,128,64,32 ; tree2 w: 2048,1024,512,256,128
TREE1_ENG = {2048: "D", 1024: "G", 512: "V", 256: "V", 128: "V", 64: "V", 32: "V"}
TREE2_ENG = {2048: "D", 1024: "V", 512: "V", 256: "V", 128: "V"}


def _legalize_waits(nc):
    """walrus rejects >1 sync wait per instruction; hoist extras onto NoOps."""
    n = 0
    for fn in nc.m.functions:
        for bb in fn.blocks:
            insts = bb.instructions
            out = []
            changed = False
            for inst in insts:
                si = inst.sync_info
                if si is not None and len(si.on_wait) > 1:
                    waits = list(si.on_wait)
                    for w in waits[:-1]:
                        n += 1
                        out.append(
                            mybir.InstNoOp(
                                name=f"I-lw-{n}",
                                engine=inst.engine,
                                sync_info=bass_rust.SyncInfo(on_wait=[w], on_update=[]),
                            )
                        )
                    inst.sync_info = bass_rust.SyncInfo(
                        on_wait=[waits[-1]], on_update=list(si.on_update)
                    )
                    changed = True
                out.append(inst)
            if changed:
                bb.instructions = out
    return n


def _build_nc():
    nc = bass.Bass()

    fT_d = nc.declare_dram_parameter("fT", [128, NT * NCHUNK * 128], CDT, isOutput=False)
    pfT_d = nc.declare_dram_parameter("pfT", [128, NT * NCHUNK * 128], CDT, isOutput=False)
    r16_sh = nc.declare_dram_parameter("r16_sh", [SHARD, D], CDT, isOutput=False)
    rhs_f = nc.declare_dram_parameter("rhs_f", [128, NCHUNK * 160], CDT, isOutput=False)
    bias_f = nc.declare_dram_parameter("bias_f", [1, 160], CDT, isOutput=False)
    rhs_pf = nc.declare_dram_parameter("rhs_pf", [128, NCHUNK * LOW], CDT, isOutput=False)
    bias_pf = nc.declare_dram_parameter("bias_pf", [LOW, 1], F32, isOutput=False)
    wp = nc.declare_dram_parameter("wp", [LOW, 2 * DIM], CDT, isOutput=False)
    fin1 = nc.declare_dram_parameter("fin1", [LOW, D], CDT, isOutput=False)
    b2w = nc.declare_dram_parameter("b2w", [MID, LOW], CDT, isOutput=False)
    out_sh = nc.declare_dram_parameter("out_sh", [SHARD, D], CDT, isOutput=True)

    Relu = mybir.ActivationFunctionType.Relu
    Ident = mybir.ActivationFunctionType.Identity

    with tile.TileContext(nc) as tc:
        with (
            tc.tile_pool(name="wpool", bufs=1) as wpool,
            tc.tile_pool(name="main", bufs=3) as main,
            tc.tile_pool(name="prod", bufs=2) as prodp,
            tc.tile_pool(name="parsb", bufs=3) as parsb,
            tc.tile_pool(name="small", bufs=3) as small,
            tc.tile_pool(name="outp", bufs=2) as outp,
            tc.tile_pool(name="lowps", bufs=2, space="PSUM") as lowps,
            tc.tile_pool(name="pst", bufs=1, space="PSUM") as pst,
            tc.tile_pool(name="parps", bufs=2, space="PSUM") as parps,
            tc.tile_pool(name="outps", bufs=1, space="PSUM") as outps,
        ):
            # ---- one-time constants / weights ----
            ident_c = wpool.tile([128, 128], CDT)
            make_identity(nc, ident_c)
            ones_row = wpool.tile([1, 160], CDT)
            nc.gpsimd.memset(ones_row, 1.0)

            rhs_f_sb = wpool.tile([128, NCHUNK, 160], CDT)
            rhs_pf_sb = wpool.tile([128, NCHUNK, LOW], CDT)
            bias_f_sb = wpool.tile([1, 160], CDT)
            bias_pf_sb = wpool.tile([LOW, 1], F32)
            wp_sb = wpool.tile([LOW, 2 * DIM], CDT)
            fin1_sb = wpool.tile([LOW, D], CDT)
            b2w_sb = wpool.tile([MID, LOW], CDT)

            def load_tile(t):
                row = slice(t * TILE_B, (t + 1) * TILE_B)
                col = slice(t * NCHUNK * 128, (t + 1) * NCHUNK * 128)
                fT = main.tile([128, NCHUNK, 128], CDT, tag="fT")
                nc.sync.dma_start(fT.rearrange("p c b -> p (c b)"), fT_d[:, col])
                pfT = main.tile([128, NCHUNK, 128], CDT, tag="pfT")
                nc.sync.dma_start(pfT.rearrange("p c b -> p (c b)"), pfT_d[:, col])
                r16 = main.tile([128, D], CDT, tag="r16")
                nc.scalar.dma_start(r16, r16_sh[row, :])
                return row, fT, pfT, r16

            def low_phase(loaded):
                row, fT, pfT, r16 = loaded
                ps_fl = lowps.tile([128, 160], F32, tag="low")
                for c in range(NCHUNK):
                    nc.tensor.matmul(
                        ps_fl, lhsT=fT[:, c, :], rhs=rhs_f_sb[:, c, :],
                        start=(c == 0), stop=False,
                    )
                nc.tensor.matmul(
                    ps_fl, lhsT=ones_row[:, :128], rhs=bias_f_sb,
                    start=False, stop=True,
                )
                # f_dup[:, 2l] = f_dup[:, 2l+1] = f_low[:, l]  (pair-dup bf16)
                f_dup = small.tile([128, 2 * LOW], CDT, tag="fdup")
                nc.scalar.activation(
                    f_dup.rearrange("p (l two) -> p l two", two=2),
                    ps_fl[:, :LOW].unsqueeze(2).broadcast_to([128, LOW, 2]),
                    Ident, bias=0.0,
                )
                h_bias = small.tile([128, MID], F32, tag="hbias")
                nc.scalar.copy(h_bias, ps_fl[:, LOW:])

                ps_pl = lowps.tile([128, 128], F32, tag="low")
                for c in range(NCHUNK):
                    nc.tensor.matmul(
                        ps_pl, lhsT=rhs_pf_sb[:, c, :], rhs=pfT[:, c, :],
                        start=(c == 0), stop=(c == NCHUNK - 1),
                    )
                pf_lowT = small.tile([128, 128], CDT, tag="pflT")
                nc.scalar.activation(pf_lowT, ps_pl, Ident, bias=bias_pf_sb)
                return row, r16, f_dup, h_bias, pf_lowT

            def params_half(pf_lowT, half, dup_sb, routes, prod_tag):
                """One params half (4096 cols) times the pair-duplicated
                activation; returns prod tile [128, 4096]."""
                prod = prodp.tile([128, DIM], CDT, tag=prod_tag)
                base = half * DIM
                if half == 0:
                    # p1 cols (l, m): chunk c = l in [32c, 32c+32)
                    nsub, sub = 32, 16
                else:
                    # p2 cols (m, l): chunk c = m in [8c, 8c+8)
                    nsub, sub = 8, 64
                for c in range(4):
                    ps = parps.tile([128, 1024], F32, tag="par")
                    for k in range(2):
                        sl = slice(base + c * 1024 + k * 512, base + c * 1024 + (k + 1) * 512)
                        nc.tensor.matmul(
                            ps[:, k * 512:(k + 1) * 512], lhsT=pf_lowT,
                            rhs=wp_sb[:, sl], start=True, stop=True,
                        )
                    dup = dup_sb[:, 2 * nsub * c: 2 * nsub * (c + 1)]
                    route = routes[c]
                    if route == "V":
                        o3 = prod[:, c * 1024:(c + 1) * 1024].rearrange(
                            "p (a b) -> p a b", a=nsub)
                        nc.vector.tensor_mul(
                            o3, ps.rearrange("p (a b) -> p a b", a=nsub),
                            dup.rearrange("p (a two) -> p a two", two=2)[:, :, 0]
                            .unsqueeze(2).broadcast_to([128, nsub, sub * 2]),
                        )
                    else:
                        pcp = parsb.tile([128, 1024], CDT, tag="pcp")
                        nc.scalar.copy(pcp, ps)
                        o4 = prod[:, c * 1024:(c + 1) * 1024].rearrange(
                            "p (a b two) -> p a b two", a=nsub, two=2)
                        i4 = pcp.rearrange("p (a b two) -> p a b two", a=nsub, two=2)
                        d4 = dup.rearrange("p (a two) -> p a two", two=2) \
                            .unsqueeze(2).broadcast_to([128, nsub, sub, 2])
                        if route == "SV":
                            nc.vector.tensor_mul(o4, i4, d4)
                        else:
                            nc.gpsimd.tensor_mul(o4, i4, d4)
                return prod

            def tree(prod, total, final, engines):
                """In-place contiguous halving reduce prod[:, :total] ->
                [:, :final]; per-level engine from `engines`."""
                w = total // 2
                while w >= final:
                    dst = prod[:, 0:w]
                    src = prod[:, w:2 * w]
                    eng = engines[w]
                    if eng == "V":
                        nc.vector.tensor_add(dst, dst, src)
                    elif eng == "G":
                        nc.gpsimd.tensor_add(dst, dst, src)
                    elif eng == "D":
                        nc.gpsimd.dma_start(dst, src, accum_op=mybir.AluOpType.add)
                    else:  # "VG": split across DVE and GPSIMD
                        hw_ = w // 2
                        nc.vector.tensor_add(
                            prod[:, 0:hw_], prod[:, 0:hw_], prod[:, w:w + hw_])
                        nc.gpsimd.tensor_add(
                            prod[:, hw_:w], prod[:, hw_:w], prod[:, w + hw_:2 * w])
                    w //= 2

            def stage_a(lowstate):
                row, r16, f_dup, h_bias, pf_lowT = lowstate
                prod1 = params_half(pf_lowT, 0, f_dup, ROUTE1, "prod1")
                tree(prod1, DIM, MID, TREE1_ENG)
                h16 = small.tile([128, MID], F32, tag="h16")
                nc.vector.tensor_add(h16, prod1[:, 0:MID], h_bias)
                # pair-duplicated relu(h)
                h_dup = small.tile([128, 2 * MID], CDT, tag="hdup")
                nc.scalar.activation(
                    h_dup.rearrange("p (m two) -> p m two", two=2),
                    h16.unsqueeze(2).broadcast_to([128, MID, 2]),
                    Relu,
                )
                prod2 = params_half(pf_lowT, 1, h_dup, ROUTE2, "prod2")
                tree(prod2, DIM, LOW, TREE2_ENG)
                return row, r16, h_dup, prod2

            def stage_b(state):
                row, r16, h_dup, prod2 = state
                ps_ht = pst.tile([MID, 128], CDT, tag="pst")
                nc.tensor.transpose(
                    ps_ht,
                    h_dup.rearrange("p (m two) -> p m two", two=2)[:, :, 0],
                    ident_c,
                )
                hT_sb = small.tile([MID, 128], CDT, tag="hTe")
                nc.scalar.copy(hT_sb, ps_ht)
                # g' = g + h @ B2  (residual path folded into g)
                ps_gx = pst.tile([128, 128], F32, tag="pst")
                nc.tensor.matmul(ps_gx, lhsT=hT_sb, rhs=b2w_sb, start=True, stop=True)
                gx_sb = small.tile([128, 128], CDT, tag="gxs")
                nc.vector.tensor_add(gx_sb, ps_gx, prod2[:, 0:LOW])
                ps_gt = pst.tile([128, 128], CDT, tag="pst")
                nc.tensor.transpose(ps_gt, gx_sb, ident_c)
                gT_sb = small.tile([128, 128], CDT, tag="gTs")
                nc.scalar.copy(gT_sb, ps_gt)

                out16 = outp.tile([128, D], CDT, tag="o16")
                for hf in range(2):
                    sl = slice(hf * 512, (hf + 1) * 512)
                    ps_o = outps.tile([128, 512], F32, tag="out")
                    nc.tensor.matmul(
                        ps_o, lhsT=gT_sb, rhs=fin1_sb[:, sl],
                        start=True, stop=True,
                    )
                    nc.vector.tensor_add(out16[:, sl], ps_o, r16[:, sl])
                nc.scalar.dma_start(out_sh[row, :], out16)

            nc.sync.dma_start(rhs_f_sb.rearrange("p c n -> p (c n)"), rhs_f[:, :])
            nc.sync.dma_start(rhs_pf_sb.rearrange("p c n -> p (c n)"), rhs_pf[:, :])
            nc.sync.dma_start(bias_f_sb, bias_f[:, :])
            nc.sync.dma_start(bias_pf_sb, bias_pf[:, :])
            loads = [load_tile(0), load_tile(1)]
            nc.sync.dma_start(wp_sb, wp[:, :])
            nc.sync.dma_start(fin1_sb, fin1[:, :])
            nc.sync.dma_start(b2w_sb, b2w[:, :])

            pending = []
            for t in range(NT):
                if t + 2 < NT:
                    loads.append(load_tile(t + 2))
                st = stage_a(low_phase(loads.pop(0)))
                pending.append(st)
                if len(pending) > 1:
                    stage_b(pending.pop(0))
            for st in pending:
                stage_b(st)

    _legalize_waits(nc)
    return nc


def _host_prep(proj_f_w, proj_f_b, proj_pf_w, proj_pf_b, proj_f2_w, proj_f2_b,
               pg_w, pg_b):
    B1 = pg_b[:DIM].reshape(LOW, MID)
    B2 = pg_b[DIM:].reshape(MID, LOW)
    c = np.ascontiguousarray
    return {
        "rhs_f": c(np.concatenate([proj_f_w.T, proj_f_w.T @ B1], axis=1)
                   .reshape(NCHUNK, 128, 160).transpose(1, 0, 2)
                   .reshape(128, NCHUNK * 160).astype(NP_CDT)),
        "bias_f": c(np.concatenate([proj_f_b, proj_f_b @ B1])[None, :].astype(NP_CDT)),
        "rhs_pf": c(proj_pf_w.T.reshape(NCHUNK, 128, LOW).transpose(1, 0, 2)
                    .reshape(128, NCHUNK * LOW).astype(NP_CDT)),
        "bias_pf": c(proj_pf_b[:, None].astype(np.float32)),
        "wp": c(pg_w.T.astype(NP_CDT)),
        "fin1": c(proj_f2_w.T.astype(NP_CDT)),
        "b2w": c(B2.astype(NP_CDT)),
    }


def _host_transpose(x16):
    """[SHARD, D] bf16 -> [128, NT*NCHUNK*128] tile-major transposed layout."""
    return np.ascontiguousarray(
        x16.reshape(NT, TILE_B, NCHUNK, 128).transpose(3, 0, 2, 1)
        .reshape(128, NT * NCHUNK * 128))


def kernel(f, pf, proj_f_w, proj_f_b, proj_pf_w, proj_pf_b, proj_f2_w, proj_f2_b,
           pg_w, pg_b):
    f = np.ascontiguousarray(np.asarray(f, dtype=np.float32))
    pf = np.ascontiguousarray(np.asarray(pf, dtype=np.float32))
    f16 = f.astype(NP_CDT)
    pf16 = pf.astype(NP_CDT)
    r16 = (f + pf + np.asarray(proj_f2_b, np.float32)[None, :]).astype(NP_CDT)
    weights = _host_prep(
        np.asarray(proj_f_w, np.float32), np.asarray(proj_f_b, np.float32),
        np.asarray(proj_pf_w, np.float32), np.asarray(proj_pf_b, np.float32),
        np.asarray(proj_f2_w, np.float32), np.asarray(proj_f2_b, np.float32),
        np.asarray(pg_w, np.float32), np.asarray(pg_b, np.float32),
    )

    if "nc" not in _CACHED:
        _CACHED["nc"] = _build_nc()
    nc = _CACHED["nc"]

    in_maps = []
    for i in range(N_CORES):
        m = dict(weights)
        sl = slice(i * SHARD, (i + 1) * SHARD)
        m["fT"] = _host_transpose(f16[sl])
        m["pfT"] = _host_transpose(pf16[sl])
        m["r16_sh"] = r16[sl]
        in_maps.append(m)

    res = run_bass_kernel_spmd(nc, in_maps, core_ids=list(range(N_CORES)))
    out = np.concatenate(
        [res.results[i]["out_sh"].astype(np.float32) for i in range(N_CORES)], axis=0
    )
    return out


# revision 8
# speedup vs baseline: 1.0809x; 1.0809x over previous
"""Trainium2 Bass kernel for nn_DynamicFc (per-sample dynamic MLP).

Data-parallel over 8 cores (batch 8192 -> 8 x 1024), 8 tiles of 128/core.

v2: host-side transposed input layouts (plain dense DMA loads, no xbar
transpose), CCE-accumulate DMA absorbs the widest tree-reduce levels,
chunk routing across DVE/Scalar/GPSIMD retuned.

Per 128-sample tile (natural [batch, feat] layout):
  psum_fl = f @ [Wf.T | Wf.T@B1] + [bf | bf@B1]        (PE, 160 cols)
  pf_lowT = (Wpf.T).T-chunks @ pfT-chunks + bpf        (PE, transposed [j,b])
  p1 = pf_low @ W1 (cols (l,m)), p2 = pf_low @ W2 (cols (m,l))   (PE)
  prod1 = p1 * f_low  (pair-duplicated broadcast; per-chunk routes)
  h = relu(tree_reduce_l(prod1) + h_bias)              (DMA/DVE/GPS tree)
  prod2 = p2 * h; g = tree_reduce_m(prod2)             (same scheme)
  out = gT.T @ fin1 + [hT].T @ B2 + r16                (PE + DVE adds)
Residual r16 = (f+pf+b2) in bf16 from host; output stored bf16.
"""

import os
import sys

import numpy as np

for _p in ("/opt/trn_rl_repo", "/root/.axon_site/_ro/trn_rl_repo"):
    if os.path.isdir(_p) and _p not in sys.path:
        sys.path.insert(0, _p)

import bass_rust
import concourse.bass as bass
import concourse.mybir as mybir
import concourse.tile as tile
from concourse.bass_utils import run_bass_kernel_spmd
from concourse.masks import make_identity

import ml_dtypes

N_CORES = 8
B, D, LOW, MID = 8192, 1024, 128, 32
DIM = LOW * MID  # 4096
SHARD = B // N_CORES  # 1024
TILE_B = 128
NT = SHARD // TILE_B  # 8
NCHUNK = D // 128  # 8

F32 = mybir.dt.float32
CDT = mybir.dt.bfloat16
NP_CDT = ml_dtypes.bfloat16

_CACHED = {}

# per-chunk routing for the params first-touch multiply (8 chunks of 512
# per half): "V" = DVE direct from psum, "SV" = scalar copy psum->sbuf bf16
# then DVE multiply, "SG" = scalar copy then gpsimd multiply.
ROUTE1 = ("V", "V", "V", "V", "SV", "SV", "SG", "SG")
ROUTE2 = ("V", "V", "V", "V", "SV", "SV", "SG", "SG")
# tree level engine by fold half-width w (contiguous halving folds):
# tree1 w: 2048,1024,512,256,128,64,32 ; tree2 w: 2048,1024,512,256,128
TREE1_ENG = {2048: "VG", 1024: "V", 512: "V", 256: "V", 128: "V", 64: "V", 32: "V"}
TREE2_ENG = {2048: "VG", 1024: "V", 512: "V", 256: "V", 128: "V"}


def _legalize_waits(nc):
    """walrus rejects >1 sync wait per instruction; hoist extras onto NoOps."""
    n = 0
    for fn in nc.m.functions:
        for bb in fn.blocks:
            insts = bb.instructions
            out = []
            changed = False
            for inst in insts:
                si = inst.sync_info
                if si is not None and len(si.on_wait) > 1:
                    waits = list(si.on_wait)
                    for w in waits[:-1]:
                        n += 1
                        out.append(
                            mybir.InstNoOp(
                                name=f"I-lw-{n}",
                                engine=inst.engine,
                                sync_info=bass_rust.SyncInfo(on_wait=[w], on_update=[]),
                            )
                        )
                    inst.sync_info = bass_rust.SyncInfo(
                        on_wait=[waits[-1]], on_update=list(si.on_update)
                    )
                    changed = True
                out.append(inst)
            if changed:
                bb.instructions = out
    return n


def _build_nc():
    nc = bass.Bass()

    fT_d = nc.declare_dram_parameter("fT", [128, NT * NCHUNK * 128], CDT, isOutput=False)
    pfT_d = nc.declare_dram_parameter("pfT", [128, NT * NCHUNK * 128], CDT, isOutput=False)
    r16_sh = nc.declare_dram_parameter("r16_sh", [SHARD, D], CDT, isOutput=False)
    rhs_f = nc.declare_dram_parameter("rhs_f", [128, NCHUNK * 160], CDT, isOutput=False)
    bias_f = nc.declare_dram_parameter("bias_f", [1, 160], CDT, isOutput=False)
    rhs_pf = nc.declare_dram_parameter("rhs_pf", [128, NCHUNK * LOW], CDT, isOutput=False)
    bias_pf = nc.declare_dram_parameter("bias_pf", [LOW, 1], F32, isOutput=False)
    wp = nc.declare_dram_parameter("wp", [LOW, 2 * DIM], CDT, isOutput=False)
    fin1 = nc.declare_dram_parameter("fin1", [LOW, D], CDT, isOutput=False)
    b2w = nc.declare_dram_parameter("b2w", [MID, LOW], CDT, isOutput=False)
    out_sh = nc.declare_dram_parameter("out_sh", [SHARD, D], CDT, isOutput=True)

    Relu = mybir.ActivationFunctionType.Relu
    Ident = mybir.ActivationFunctionType.Identity

    with tile.TileContext(nc) as tc:
        with (
            tc.tile_pool(name="wpool", bufs=1) as wpool,
            tc.tile_pool(name="main", bufs=3) as main,
            tc.tile_pool(name="prod", bufs=2) as prodp,
            tc.tile_pool(name="parsb", bufs=3) as parsb,
            tc.tile_pool(name="small", bufs=3) as small,
            tc.tile_pool(name="outp", bufs=2) as outp,
            tc.tile_pool(name="lowps", bufs=2, space="PSUM") as lowps,
            tc.tile_pool(name="pst", bufs=1, space="PSUM") as pst,
            tc.tile_pool(name="parps", bufs=4, space="PSUM") as parps,
            tc.tile_pool(name="outps", bufs=1, space="PSUM") as outps,
        ):
            # ---- one-time constants / weights ----
            ident_c = wpool.tile([128, 128], CDT)
            make_identity(nc, ident_c)
            ones_row = wpool.tile([1, 160], CDT)
            nc.gpsimd.memset(ones_row, 1.0)

            rhs_f_sb = wpool.tile([128, NCHUNK, 160], CDT)
            rhs_pf_sb = wpool.tile([128, NCHUNK, LOW], CDT)
            bias_f_sb = wpool.tile([1, 160], CDT)
            bias_pf_sb = wpool.tile([LOW, 1], F32)
            wp_sb = wpool.tile([LOW, 2 * DIM], CDT)
            fin1_sb = wpool.tile([LOW, D], CDT)
            b2w_sb = wpool.tile([MID, LOW], CDT)

            def load_tile(t):
                row = slice(t * TILE_B, (t + 1) * TILE_B)
                col = slice(t * NCHUNK * 128, (t + 1) * NCHUNK * 128)
                fT = main.tile([128, NCHUNK, 128], CDT, tag="fT")
                nc.sync.dma_start(fT.rearrange("p c b -> p (c b)"), fT_d[:, col])
                pfT = main.tile([128, NCHUNK, 128], CDT, tag="pfT")
                nc.sync.dma_start(pfT.rearrange("p c b -> p (c b)"), pfT_d[:, col])
                r16 = main.tile([128, D], CDT, tag="r16")
                nc.sync.dma_start(r16, r16_sh[row, :])
                return row, fT, pfT, r16

            def low_phase(loaded):
                row, fT, pfT, r16 = loaded
                ps_fl = lowps.tile([128, 160], F32, tag="low")
                for c in range(NCHUNK):
                    nc.tensor.matmul(
                        ps_fl, lhsT=fT[:, c, :], rhs=rhs_f_sb[:, c, :],
                        start=(c == 0), stop=False,
                    )
                nc.tensor.matmul(
                    ps_fl, lhsT=ones_row[:, :128], rhs=bias_f_sb,
                    start=False, stop=True,
                )
                # f_dup[:, 2l] = f_dup[:, 2l+1] = f_low[:, l]  (pair-dup bf16)
                f_dup = small.tile([128, 2 * LOW], CDT, tag="fdup")
                nc.scalar.activation(
                    f_dup.rearrange("p (l two) -> p l two", two=2),
                    ps_fl[:, :LOW].unsqueeze(2).broadcast_to([128, LOW, 2]),
                    Ident, bias=0.0,
                )
                h_bias = small.tile([128, MID], F32, tag="hbias")
                nc.scalar.copy(h_bias, ps_fl[:, LOW:])

                ps_pl = lowps.tile([128, 128], F32, tag="low")
                for c in range(NCHUNK):
                    nc.tensor.matmul(
                        ps_pl, lhsT=rhs_pf_sb[:, c, :], rhs=pfT[:, c, :],
                        start=(c == 0), stop=(c == NCHUNK - 1),
                    )
                pf_lowT = small.tile([128, 128], CDT, tag="pflT")
                nc.scalar.activation(pf_lowT, ps_pl, Ident, bias=bias_pf_sb)
                return row, r16, f_dup, h_bias, pf_lowT

            def params_half(pf_lowT, half, dup_sb, routes, prod_tag):
                """One params half (4096 cols, 8 chunks of 512) times the
                pair-duplicated activation; returns prod tile [128, 4096]."""
                prod = prodp.tile([128, DIM], CDT, tag=prod_tag)
                base = half * DIM
                if half == 0:
                    # p1 cols (l, m): 512-chunk c = l in [16c, 16c+16)
                    nsub, sub = 16, 16
                else:
                    # p2 cols (m, l): 512-chunk c = m in [4c, 4c+4)
                    nsub, sub = 4, 64
                for c in range(8):
                    ps = parps.tile([128, 512], F32, tag="par")
                    sl = slice(base + c * 512, base + (c + 1) * 512)
                    nc.tensor.matmul(
                        ps, lhsT=pf_lowT, rhs=wp_sb[:, sl], start=True, stop=True,
                    )
                    dup = dup_sb[:, 2 * nsub * c: 2 * nsub * (c + 1)]
                    route = routes[c]
                    if route == "V":
                        o3 = prod[:, c * 512:(c + 1) * 512].rearrange(
                            "p (a b) -> p a b", a=nsub)
                        nc.vector.tensor_mul(
                            o3, ps.rearrange("p (a b) -> p a b", a=nsub),
                            dup.rearrange("p (a two) -> p a two", two=2)[:, :, 0]
                            .unsqueeze(2).broadcast_to([128, nsub, sub * 2]),
                        )
                    else:
                        pcp = parsb.tile([128, 512], CDT, tag="pcp")
                        nc.scalar.copy(pcp, ps)
                        o4 = prod[:, c * 512:(c + 1) * 512].rearrange(
                            "p (a b two) -> p a b two", a=nsub, two=2)
                        i4 = pcp.rearrange("p (a b two) -> p a b two", a=nsub, two=2)
                        d4 = dup.rearrange("p (a two) -> p a two", two=2) \
                            .unsqueeze(2).broadcast_to([128, nsub, sub, 2])
                        if route == "SV":
                            nc.vector.tensor_mul(o4, i4, d4)
                        else:
                            nc.gpsimd.tensor_mul(o4, i4, d4)
                return prod

            def tree(prod, total, final, engines):
                """In-place contiguous halving reduce prod[:, :total] ->
                [:, :final]; per-level engine from `engines`."""
                w = total // 2
                while w >= final:
                    dst = prod[:, 0:w]
                    src = prod[:, w:2 * w]
                    eng = engines[w]
                    if eng == "V":
                        nc.vector.tensor_add(dst, dst, src)
                    elif eng == "G":
                        nc.gpsimd.tensor_add(dst, dst, src)
                    elif eng == "D":
                        nc.gpsimd.dma_start(dst, src, accum_op=mybir.AluOpType.add)
                    else:  # "VG": split across DVE and GPSIMD
                        hw_ = w // 2
                        nc.vector.tensor_add(
                            prod[:, 0:hw_], prod[:, 0:hw_], prod[:, w:w + hw_])
                        nc.gpsimd.tensor_add(
                            prod[:, hw_:w], prod[:, hw_:w], prod[:, w + hw_:2 * w])
                    w //= 2

            def stage_a(lowstate):
                row, r16, f_dup, h_bias, pf_lowT = lowstate
                prod1 = params_half(pf_lowT, 0, f_dup, ROUTE1, "prod1")
                tree(prod1, DIM, MID, TREE1_ENG)
                h16 = small.tile([128, MID], F32, tag="h16")
                nc.vector.tensor_add(h16, prod1[:, 0:MID], h_bias)
                # pair-duplicated relu(h)
                h_dup = small.tile([128, 2 * MID], CDT, tag="hdup")
                nc.scalar.activation(
                    h_dup.rearrange("p (m two) -> p m two", two=2),
                    h16.unsqueeze(2).broadcast_to([128, MID, 2]),
                    Relu,
                )
                prod2 = params_half(pf_lowT, 1, h_dup, ROUTE2, "prod2")
                tree(prod2, DIM, LOW, TREE2_ENG)
                return row, r16, h_dup, prod2

            def stage_b(state):
                row, r16, h_dup, prod2 = state
                ps_ht = pst.tile([MID, 128], CDT, tag="pst")
                nc.tensor.transpose(
                    ps_ht,
                    h_dup.rearrange("p (m two) -> p m two", two=2)[:, :, 0],
                    ident_c,
                )
                hT_sb = small.tile([MID, 128], CDT, tag="hTe")
                nc.scalar.copy(hT_sb, ps_ht)
                # g' = g + h @ B2  (residual path folded into g)
                ps_gx = pst.tile([128, 128], F32, tag="pst")
                nc.tensor.matmul(ps_gx, lhsT=hT_sb, rhs=b2w_sb, start=True, stop=True)
                gx_sb = small.tile([128, 128], CDT, tag="gxs")
                nc.vector.tensor_add(gx_sb, ps_gx, prod2[:, 0:LOW])
                ps_gt = pst.tile([128, 128], CDT, tag="pst")
                nc.tensor.transpose(ps_gt, gx_sb, ident_c)
                gT_sb = small.tile([128, 128], CDT, tag="gTs")
                nc.scalar.copy(gT_sb, ps_gt)

                out16 = outp.tile([128, D], CDT, tag="o16")
                for hf in range(2):
                    sl = slice(hf * 512, (hf + 1) * 512)
                    ps_o = outps.tile([128, 512], F32, tag="out")
                    nc.tensor.matmul(
                        ps_o, lhsT=gT_sb, rhs=fin1_sb[:, sl],
                        start=True, stop=True,
                    )
                    nc.vector.tensor_add(out16[:, sl], ps_o, r16[:, sl])
                nc.sync.dma_start(out_sh[row, :], out16)

            nc.sync.dma_start(rhs_f_sb.rearrange("p c n -> p (c n)"), rhs_f[:, :])
            nc.sync.dma_start(rhs_pf_sb.rearrange("p c n -> p (c n)"), rhs_pf[:, :])
            nc.sync.dma_start(bias_f_sb, bias_f[:, :])
            nc.sync.dma_start(bias_pf_sb, bias_pf[:, :])
            loads = [load_tile(0), load_tile(1)]
            nc.sync.dma_start(wp_sb, wp[:, :])
            nc.sync.dma_start(fin1_sb, fin1[:, :])
            nc.sync.dma_start(b2w_sb, b2w[:, :])

            pending = []
            for t in range(NT):
                if t + 2 < NT:
                    loads.append(load_tile(t + 2))
                st = stage_a(low_phase(loads.pop(0)))
                pending.append(st)
                if len(pending) > 1:
                    stage_b(pending.pop(0))
            for st in pending:
                stage_b(st)

    _legalize_waits(nc)
    return nc


def _host_prep(proj_f_w, proj_f_b, proj_pf_w, proj_pf_b, proj_f2_w, proj_f2_b,
               pg_w, pg_b):
    B1 = pg_b[:DIM].reshape(LOW, MID)
    B2 = pg_b[DIM:].reshape(MID, LOW)
    c = np.ascontiguousarray
    return {
        "rhs_f": c(np.concatenate([proj_f_w.T, proj_f_w.T @ B1], axis=1)
                   .reshape(NCHUNK, 128, 160).transpose(1, 0, 2)
                   .reshape(128, NCHUNK * 160).astype(NP_CDT)),
        "bias_f": c(np.concatenate([proj_f_b, proj_f_b @ B1])[None, :].astype(NP_CDT)),
        "rhs_pf": c(proj_pf_w.T.reshape(NCHUNK, 128, LOW).transpose(1, 0, 2)
                    .reshape(128, NCHUNK * LOW).astype(NP_CDT)),
        "bias_pf": c(proj_pf_b[:, None].astype(np.float32)),
        "wp": c(pg_w.T.astype(NP_CDT)),
        "fin1": c(proj_f2_w.T.astype(NP_CDT)),
        "b2w": c(B2.astype(NP_CDT)),
    }


def _host_transpose(x16):
    """[SHARD, D] bf16 -> [128, NT*NCHUNK*128] tile-major transposed layout."""
    return np.ascontiguousarray(
        x16.reshape(NT, TILE_B, NCHUNK, 128).transpose(3, 0, 2, 1)
        .reshape(128, NT * NCHUNK * 128))


def kernel(f, pf, proj_f_w, proj_f_b, proj_pf_w, proj_pf_b, proj_f2_w, proj_f2_b,
           pg_w, pg_b):
    f = np.ascontiguousarray(np.asarray(f, dtype=np.float32))
    pf = np.ascontiguousarray(np.asarray(pf, dtype=np.float32))
    f16 = f.astype(NP_CDT)
    pf16 = pf.astype(NP_CDT)
    r16 = (f + pf + np.asarray(proj_f2_b, np.float32)[None, :]).astype(NP_CDT)
    weights = _host_prep(
        np.asarray(proj_f_w, np.float32), np.asarray(proj_f_b, np.float32),
        np.asarray(proj_pf_w, np.float32), np.asarray(proj_pf_b, np.float32),
        np.asarray(proj_f2_w, np.float32), np.asarray(proj_f2_b, np.float32),
        np.asarray(pg_w, np.float32), np.asarray(pg_b, np.float32),
    )

    if "nc" not in _CACHED:
        _CACHED["nc"] = _build_nc()
    nc = _CACHED["nc"]

    in_maps = []
    for i in range(N_CORES):
        m = dict(weights)
        sl = slice(i * SHARD, (i + 1) * SHARD)
        m["fT"] = _host_transpose(f16[sl])
        m["pfT"] = _host_transpose(pf16[sl])
        m["r16_sh"] = r16[sl]
        in_maps.append(m)

    res = run_bass_kernel_spmd(nc, in_maps, core_ids=list(range(N_CORES)))
    out = np.concatenate(
        [res.results[i]["out_sh"].astype(np.float32) for i in range(N_CORES)], axis=0
    )
    return out


# revision 14
# speedup vs baseline: 1.2187x; 1.1275x over previous
"""Trainium2 Bass kernel for nn_DynamicFc (per-sample dynamic MLP).

Data-parallel over 8 cores (batch 8192 -> 8 x 1024), 8 tiles of 128/core.

v2: host-side transposed input layouts (plain dense DMA loads, no xbar
transpose), CCE-accumulate DMA absorbs the widest tree-reduce levels,
chunk routing across DVE/Scalar/GPSIMD retuned.

Per 128-sample tile (natural [batch, feat] layout):
  psum_fl = f @ [Wf.T | Wf.T@B1] + [bf | bf@B1]        (PE, 160 cols)
  pf_lowT = (Wpf.T).T-chunks @ pfT-chunks + bpf        (PE, transposed [j,b])
  p1 = pf_low @ W1 (cols (l,m)), p2 = pf_low @ W2 (cols (m,l))   (PE)
  prod1 = p1 * f_low  (pair-duplicated broadcast; per-chunk routes)
  h = relu(tree_reduce_l(prod1) + h_bias)              (DMA/DVE/GPS tree)
  prod2 = p2 * h; g = tree_reduce_m(prod2)             (same scheme)
  out = gT.T @ fin1 + [hT].T @ B2 + r16                (PE + DVE adds)
Residual r16 = (f+pf+b2) in bf16 from host; output stored bf16.
"""

import os
import sys

import numpy as np

for _p in ("/opt/trn_rl_repo", "/root/.axon_site/_ro/trn_rl_repo"):
    if os.path.isdir(_p) and _p not in sys.path:
        sys.path.insert(0, _p)

import bass_rust
import concourse.bass as bass
import concourse.mybir as mybir
import concourse.tile as tile
from concourse.bass_utils import run_bass_kernel_spmd
from concourse.masks import make_identity

import ml_dtypes

N_CORES = 8
B, D, LOW, MID = 8192, 1024, 128, 32
DIM = LOW * MID  # 4096
SHARD = B // N_CORES  # 1024
TILE_B = 128
NT = SHARD // TILE_B  # 8
NCHUNK = D // 128  # 8

F32 = mybir.dt.float32
CDT = mybir.dt.bfloat16
NP_CDT = ml_dtypes.bfloat16

_CACHED = {}

# per-chunk routing for the params first-touch multiply (8 chunks of 512
# per half): "V" = DVE direct from psum, "SV" = scalar copy psum->sbuf bf16
# then DVE multiply, "SG" = scalar copy then gpsimd multiply.
ROUTE1 = ("V", "V", "V", "V", "SV", "SV", "SG", "SG")
ROUTE2 = ("V", "V", "V", "V", "SV", "SV", "SG", "SG")
# tree level engine by fold half-width w (contiguous halving folds):
# tree1 w: 2048,1024,512,256,128,64,32 ; tree2 w: 2048,1024,512,256,128
# value: engine, or (engA, engB) to split the fold into two concurrent pieces
TREE1_ENG = {2048: ("V", "G"), 1024: "V", 512: "V", 256: "V", 128: "V",
             64: "V", 32: "V"}
TREE2_ENG = {2048: ("V", "G"), 1024: "V", 512: "V", 256: "V", 128: "V"}


def _legalize_waits(nc):
    """walrus rejects >1 sync wait per instruction; hoist extras onto NoOps."""
    n = 0
    for fn in nc.m.functions:
        for bb in fn.blocks:
            insts = bb.instructions
            out = []
            changed = False
            for inst in insts:
                si = inst.sync_info
                if si is not None and len(si.on_wait) > 1:
                    waits = list(si.on_wait)
                    for w in waits[:-1]:
                        n += 1
                        out.append(
                            mybir.InstNoOp(
                                name=f"I-lw-{n}",
                                engine=inst.engine,
                                sync_info=bass_rust.SyncInfo(on_wait=[w], on_update=[]),
                            )
                        )
                    inst.sync_info = bass_rust.SyncInfo(
                        on_wait=[waits[-1]], on_update=list(si.on_update)
                    )
                    changed = True
                out.append(inst)
            if changed:
                bb.instructions = out
    return n


def _build_nc():
    nc = bass.Bass()

    fT_d = nc.declare_dram_parameter("fT", [128, NT * NCHUNK * 128], CDT, isOutput=False)
    pfT_d = nc.declare_dram_parameter("pfT", [128, NT * NCHUNK * 128], CDT, isOutput=False)
    r16_sh = nc.declare_dram_parameter("r16_sh", [SHARD, D], CDT, isOutput=False)
    rhs_f = nc.declare_dram_parameter("rhs_f", [128, NCHUNK * 160], CDT, isOutput=False)
    bias_f = nc.declare_dram_parameter("bias_f", [1, 160], CDT, isOutput=False)
    rhs_pf = nc.declare_dram_parameter("rhs_pf", [128, NCHUNK * LOW], CDT, isOutput=False)
    bias_pf = nc.declare_dram_parameter("bias_pf", [LOW, 1], F32, isOutput=False)
    wp = nc.declare_dram_parameter("wp", [LOW, 2 * DIM], CDT, isOutput=False)
    fin1 = nc.declare_dram_parameter("fin1", [LOW, D], CDT, isOutput=False)
    b2w = nc.declare_dram_parameter("b2w", [MID, LOW], CDT, isOutput=False)
    out_sh = nc.declare_dram_parameter("out_sh", [SHARD, D], CDT, isOutput=True)

    Relu = mybir.ActivationFunctionType.Relu
    Ident = mybir.ActivationFunctionType.Identity

    with tile.TileContext(nc) as tc:
        with (
            tc.tile_pool(name="wpool", bufs=1) as wpool,
            tc.tile_pool(name="main", bufs=3) as main,
            tc.tile_pool(name="resid", bufs=3) as resid,
            tc.tile_pool(name="prod", bufs=3) as prodp,
            tc.tile_pool(name="parsb", bufs=4) as parsb,
            tc.tile_pool(name="small", bufs=6) as small,
            tc.tile_pool(name="outp", bufs=2) as outp,
            tc.tile_pool(name="lowps", bufs=2, space="PSUM") as lowps,
            tc.tile_pool(name="pst", bufs=1, space="PSUM") as pst,
            tc.tile_pool(name="parps", bufs=4, space="PSUM") as parps,
            tc.tile_pool(name="outps", bufs=1, space="PSUM") as outps,
        ):
            # ---- one-time constants / weights ----
            ident_c = wpool.tile([128, 128], CDT)
            make_identity(nc, ident_c)
            ones_row = wpool.tile([1, 160], CDT)
            nc.gpsimd.memset(ones_row, 1.0)

            rhs_f_sb = wpool.tile([128, NCHUNK, 160], CDT)
            rhs_pf_sb = wpool.tile([128, NCHUNK, LOW], CDT)
            bias_f_sb = wpool.tile([1, 160], CDT)
            bias_pf_sb = wpool.tile([LOW, 1], F32)
            wp_sb = wpool.tile([LOW, 2 * DIM], CDT)
            fin1_sb = wpool.tile([LOW, D], CDT)
            b2w_sb = wpool.tile([MID, LOW], CDT)

            def load_tile(t):
                row = slice(t * TILE_B, (t + 1) * TILE_B)
                col = slice(t * NCHUNK * 128, (t + 1) * NCHUNK * 128)
                fT = main.tile([128, NCHUNK, 128], CDT, tag="fT")
                nc.sync.dma_start(fT.rearrange("p c b -> p (c b)"), fT_d[:, col])
                pfT = main.tile([128, NCHUNK, 128], CDT, tag="pfT")
                nc.sync.dma_start(pfT.rearrange("p c b -> p (c b)"), pfT_d[:, col])
                return {"row": row, "fT": fT, "pfT": pfT}

            def low_phase(loaded):
                row, fT, pfT, _ = loaded
                ps_fl = lowps.tile([128, 160], F32, tag="low")
                for c in range(NCHUNK):
                    nc.tensor.matmul(
                        ps_fl, lhsT=fT[:, c, :], rhs=rhs_f_sb[:, c, :],
                        start=(c == 0), stop=False,
                    )
                nc.tensor.matmul(
                    ps_fl, lhsT=ones_row[:, :128], rhs=bias_f_sb,
                    start=False, stop=True,
                )
                # f_dup[:, 2l] = f_dup[:, 2l+1] = f_low[:, l]  (pair-dup bf16)
                f_dup = small.tile([128, 2 * LOW], CDT, tag="fdup")
                nc.scalar.activation(
                    f_dup.rearrange("p (l two) -> p l two", two=2),
                    ps_fl[:, :LOW].unsqueeze(2).broadcast_to([128, LOW, 2]),
                    Ident, bias=0.0,
                )
                h_bias = small.tile([128, MID], F32, tag="hbias")
                nc.scalar.copy(h_bias, ps_fl[:, LOW:])

                ps_pl = lowps.tile([128, 128], F32, tag="low")
                for c in range(NCHUNK):
                    nc.tensor.matmul(
                        ps_pl, lhsT=rhs_pf_sb[:, c, :], rhs=pfT[:, c, :],
                        start=(c == 0), stop=(c == NCHUNK - 1),
                    )
                pf_lowT = small.tile([128, 128], CDT, tag="pflT")
                nc.scalar.activation(pf_lowT, ps_pl, Ident, bias=bias_pf_sb)
                return row, None, f_dup, h_bias, pf_lowT

            def params_half(pf_lowT, half, dup_sb, routes, prod_tag):
                """One params half (4096 cols, 8 chunks of 512) times the
                pair-duplicated activation; returns prod tile [128, 4096]."""
                prod = prodp.tile([128, DIM], CDT, tag=prod_tag)
                base = half * DIM
                if half == 0:
                    # p1 cols (l, m): 512-chunk c = l in [16c, 16c+16)
                    nsub, sub = 16, 16
                else:
                    # p2 cols (m, l): 512-chunk c = m in [4c, 4c+4)
                    nsub, sub = 4, 64
                for c in range(8):
                    ps = parps.tile([128, 512], F32, tag="par")
                    sl = slice(base + c * 512, base + (c + 1) * 512)
                    nc.tensor.matmul(
                        ps, lhsT=pf_lowT, rhs=wp_sb[:, sl], start=True, stop=True,
                    )
                    dup = dup_sb[:, 2 * nsub * c: 2 * nsub * (c + 1)]
                    route = routes[c]
                    if route == "V":
                        o3 = prod[:, c * 512:(c + 1) * 512].rearrange(
                            "p (a b) -> p a b", a=nsub)
                        nc.vector.tensor_mul(
                            o3, ps.rearrange("p (a b) -> p a b", a=nsub),
                            dup.rearrange("p (a two) -> p a two", two=2)[:, :, 0]
                            .unsqueeze(2).broadcast_to([128, nsub, sub * 2]),
                        )
                    else:
                        pcp = parsb.tile([128, 512], CDT, tag="pcp")
                        nc.scalar.copy(pcp, ps)
                        o4 = prod[:, c * 512:(c + 1) * 512].rearrange(
                            "p (a b two) -> p a b two", a=nsub, two=2)
                        i4 = pcp.rearrange("p (a b two) -> p a b two", a=nsub, two=2)
                        d4 = dup.rearrange("p (a two) -> p a two", two=2) \
                            .unsqueeze(2).broadcast_to([128, nsub, sub, 2])
                        if route == "SV":
                            nc.vector.tensor_mul(o4, i4, d4)
                        else:
                            nc.gpsimd.tensor_mul(o4, i4, d4)
                return prod

            def _eng_add(eng, dst, a, b):
                if eng == "V":
                    nc.vector.tensor_add(dst, a, b)
                else:
                    nc.gpsimd.tensor_add(dst, a, b)

            def tree(prod, total, final, engines):
                """In-place contiguous halving reduce prod[:, :total] ->
                [:, :final]; per-level engine (or 2-way split) from `engines`."""
                w = total // 2
                while w >= final:
                    eng = engines[w]
                    if isinstance(eng, tuple):
                        hw_ = w // 2
                        _eng_add(eng[0], prod[:, 0:hw_], prod[:, 0:hw_],
                                 prod[:, w:w + hw_])
                        _eng_add(eng[1], prod[:, hw_:w], prod[:, hw_:w],
                                 prod[:, w + hw_:2 * w])
                    else:
                        _eng_add(eng, prod[:, 0:w], prod[:, 0:w], prod[:, w:2 * w])
                    w //= 2

            # ---- modulo-scheduled pipeline stages (one tile dict each) ----
            def s_load(st):
                st.update(load_tile(st["t"]))

            def s_low(st):
                row, r16_, f_dup, h_bias, pf_lowT = low_phase(
                    (st["row"], st["fT"], st["pfT"], None))
                st.update(f_dup=f_dup, h_bias=h_bias, pf_lowT=pf_lowT)

            def s_par1(st):
                st["prod1"] = params_half(st["pf_lowT"], 0, st["f_dup"],
                                          ROUTE1, "prod1")

            def s_tree1(st):
                prod1 = st["prod1"]
                tree(prod1, DIM, MID, TREE1_ENG)
                h16 = small.tile([128, MID], F32, tag="h16")
                nc.vector.tensor_add(h16, prod1[:, 0:MID], st["h_bias"])
                # pair-duplicated relu(h)
                h_dup = small.tile([128, 2 * MID], CDT, tag="hdup")
                nc.scalar.activation(
                    h_dup.rearrange("p (m two) -> p m two", two=2),
                    h16.unsqueeze(2).broadcast_to([128, MID, 2]),
                    Relu,
                )
                st["h_dup"] = h_dup

            def s_par2(st):
                st["prod2"] = params_half(st["pf_lowT"], 1, st["h_dup"],
                                          ROUTE2, "prod2")

            def s_tree2(st):
                tree(st["prod2"], DIM, LOW, TREE2_ENG)
                row = st["row"]
                r16 = resid.tile([128, D], CDT, tag="r16")
                nc.sync.dma_start(r16, r16_sh[row, :])
                st["r16"] = r16

            def s_out(st):
                row, r16, h_dup, prod2 = st["row"], st["r16"], st["h_dup"], st["prod2"]
                ps_ht = pst.tile([MID, 128], CDT, tag="pst")
                nc.tensor.transpose(
                    ps_ht,
                    h_dup.rearrange("p (m two) -> p m two", two=2)[:, :, 0],
                    ident_c,
                )
                hT_sb = small.tile([MID, 128], CDT, tag="hTe")
                nc.scalar.copy(hT_sb, ps_ht)
                # g' = g + h @ B2  (residual path folded into g)
                ps_gx = pst.tile([128, 128], F32, tag="pst")
                nc.tensor.matmul(ps_gx, lhsT=hT_sb, rhs=b2w_sb, start=True, stop=True)
                gx_sb = small.tile([128, 128], CDT, tag="gxs")
                nc.vector.tensor_add(gx_sb, ps_gx, prod2[:, 0:LOW])
                ps_gt = pst.tile([128, 128], CDT, tag="pst")
                nc.tensor.transpose(ps_gt, gx_sb, ident_c)
                gT_sb = small.tile([128, 128], CDT, tag="gTs")
                nc.scalar.copy(gT_sb, ps_gt)

                out16 = outp.tile([128, D], CDT, tag="o16")
                for hf in range(2):
                    sl = slice(hf * 512, (hf + 1) * 512)
                    ps_o = outps.tile([128, 512], F32, tag="out")
                    nc.tensor.matmul(
                        ps_o, lhsT=gT_sb, rhs=fin1_sb[:, sl],
                        start=True, stop=True,
                    )
                    o_cp = small.tile([128, 512], CDT, tag="ocp")
                    nc.scalar.copy(o_cp, ps_o)
                    nc.vector.tensor_add(out16[:, sl], o_cp, r16[:, sl])
                nc.sync.dma_start(out_sh[row, :], out16)

            nc.sync.dma_start(rhs_f_sb.rearrange("p c n -> p (c n)"), rhs_f[:, :])
            nc.sync.dma_start(rhs_pf_sb.rearrange("p c n -> p (c n)"), rhs_pf[:, :])
            nc.sync.dma_start(bias_f_sb, bias_f[:, :])
            nc.sync.dma_start(bias_pf_sb, bias_pf[:, :])
            nc.sync.dma_start(wp_sb, wp[:, :])
            nc.sync.dma_start(fin1_sb, fin1[:, :])
            nc.sync.dma_start(b2w_sb, b2w[:, :])

            stages = [s_load, s_low, s_par1, s_tree1, s_par2, s_tree2, s_out]
            nstages = len(stages)
            states = {t: {"t": t} for t in range(NT)}
            for step in range(NT + nstages - 1):
                for s in range(nstages - 1, -1, -1):  # deepest stage first
                    t = step - s
                    if 0 <= t < NT:
                        stages[s](states[t])

    _legalize_waits(nc)
    return nc


def _host_prep(proj_f_w, proj_f_b, proj_pf_w, proj_pf_b, proj_f2_w, proj_f2_b,
               pg_w, pg_b):
    B1 = pg_b[:DIM].reshape(LOW, MID)
    B2 = pg_b[DIM:].reshape(MID, LOW)
    c = np.ascontiguousarray
    return {
        "rhs_f": c(np.concatenate([proj_f_w.T, proj_f_w.T @ B1], axis=1)
                   .reshape(NCHUNK, 128, 160).transpose(1, 0, 2)
                   .reshape(128, NCHUNK * 160).astype(NP_CDT)),
        "bias_f": c(np.concatenate([proj_f_b, proj_f_b @ B1])[None, :].astype(NP_CDT)),
        "rhs_pf": c(proj_pf_w.T.reshape(NCHUNK, 128, LOW).transpose(1, 0, 2)
                    .reshape(128, NCHUNK * LOW).astype(NP_CDT)),
        "bias_pf": c(proj_pf_b[:, None].astype(np.float32)),
        "wp": c(pg_w.T.astype(NP_CDT)),
        "fin1": c(proj_f2_w.T.astype(NP_CDT)),
        "b2w": c(B2.astype(NP_CDT)),
    }


def _host_transpose(x16):
    """[SHARD, D] bf16 -> [128, NT*NCHUNK*128] tile-major transposed layout."""
    return np.ascontiguousarray(
        x16.reshape(NT, TILE_B, NCHUNK, 128).transpose(3, 0, 2, 1)
        .reshape(128, NT * NCHUNK * 128))


def kernel(f, pf, proj_f_w, proj_f_b, proj_pf_w, proj_pf_b, proj_f2_w, proj_f2_b,
           pg_w, pg_b):
    f = np.ascontiguousarray(np.asarray(f, dtype=np.float32))
    pf = np.ascontiguousarray(np.asarray(pf, dtype=np.float32))
    f16 = f.astype(NP_CDT)
    pf16 = pf.astype(NP_CDT)
    r16 = (f + pf + np.asarray(proj_f2_b, np.float32)[None, :]).astype(NP_CDT)
    weights = _host_prep(
        np.asarray(proj_f_w, np.float32), np.asarray(proj_f_b, np.float32),
        np.asarray(proj_pf_w, np.float32), np.asarray(proj_pf_b, np.float32),
        np.asarray(proj_f2_w, np.float32), np.asarray(proj_f2_b, np.float32),
        np.asarray(pg_w, np.float32), np.asarray(pg_b, np.float32),
    )

    if "nc" not in _CACHED:
        _CACHED["nc"] = _build_nc()
    nc = _CACHED["nc"]

    in_maps = []
    for i in range(N_CORES):
        m = dict(weights)
        sl = slice(i * SHARD, (i + 1) * SHARD)
        m["fT"] = _host_transpose(f16[sl])
        m["pfT"] = _host_transpose(pf16[sl])
        m["r16_sh"] = r16[sl]
        in_maps.append(m)

    res = run_bass_kernel_spmd(nc, in_maps, core_ids=list(range(N_CORES)))
    out = np.concatenate(
        [res.results[i]["out_sh"].astype(np.float32) for i in range(N_CORES)], axis=0
    )
    return out
